# revision 29
# baseline (speedup 1.0000x reference)
"""FeaStConv dual-branch GNN message passing on 8 Trainium2 NeuronCores.

Sharding: branch v on cores 0-3, branch f on cores 4-7; each core owns a
12500-node destination range. Host reorders edges by destination block
(64 nodes), pre-gathers transposed source/dest features (bf16), device does
all float math: per-tile matmuls for x@W / (x_s-x_d)@U, softmax on-device,
one-hot scatter matmuls accumulating per-block in PSUM.
"""
import sys, types
import numpy as np

sys.path.insert(0, '/opt/trn_rl_repo')

N = 50000
IN_CH = 64
HEADS = 4
OUT_CH = 32
P = 128
NPC = 12500           # nodes per core
BLK = 64              # dst nodes per block
NBLK = 210            # blocks per core (210*64 = 13440)
NPAD = NBLK * BLK
BLK_EDGE_CAP = 1024   # pack blocks to <= 8 tiles of edges where possible
CH = 12              # tiles per chunk
SCT = 36              # tiles per superchunk
NCORES = 8


def _register_ntff_hook():
    import antenv
    if "antenv.axon_hooks" in sys.modules:
        return
    mod = types.ModuleType("antenv.axon_hooks")
    _h = [None]
    mod.set_axon_ntff_profile_hook = lambda h: _h.__setitem__(0, h)
    mod.get_axon_ntff_profile_hook = lambda: _h[0]
    sys.modules["antenv.axon_hooks"] = mod
    antenv.axon_hooks = mod
    if "/root/.axon_site" not in sys.path:
        sys.path.insert(0, "/root/.axon_site")
    try:
        from trn_agent_boot.trn_boot import _ntff_profile_via_ctypes
        mod.set_axon_ntff_profile_hook(_ntff_profile_via_ctypes('/opt/axon/libaxon_pjrt.so'))
    except Exception:
        pass


def _pack_blocks(cnt_node):
    """Assign each of the NPC destination nodes to one of NBLK 64-node blocks,
    packing so block edge totals stay <= BLK_EDGE_CAP (8 tiles) where
    possible; overflow concentrates in the highest-index blocks so the
    max-over-cores tile profile stays tight. Returns gmap[node] -> padded
    node id (block*BLK + slot)."""
    order = np.argsort(-cnt_node, kind='stable')
    bsum = np.zeros(NBLK, np.int64)
    bcnt = np.zeros(NBLK, np.int64)
    gmap = np.empty(NPC, np.int64)
    idx = np.arange(NBLK)
    for n in order:
        c = int(cnt_node[n])
        fit = (bcnt < BLK) & (bsum + c <= BLK_EDGE_CAP)
        if fit.any():
            # best-fit: fullest block that still fits
            b = int(np.argmax(np.where(fit, bsum, -1)))
        else:
            room = bcnt < BLK
            b = int(idx[room][-1])  # spill into highest-index open block
        gmap[n] = b * BLK + bcnt[b]
        bcnt[b] += 1
        bsum[b] += c
    return gmap


def _prep_core(x16, src, dst, lo):
    """Per-core edge layout. Returns dict with per-block counts and sorted
    (global-src, packed-local-dst, original-global-dst) arrays."""
    sel = (dst >= lo) & (dst < lo + NPC)
    s = src[sel]
    d0 = (dst[sel] - lo).astype(np.int64)
    cnt_node = np.bincount(d0, minlength=NPC).astype(np.int64)
    gmap = _pack_blocks(cnt_node)
    dn = gmap[d0]
    order = np.argsort(dn, kind='stable')
    s = s[order]
    dn = dn[order]
    dor = d0[order] + lo
    cnt = np.bincount(dn >> 6, minlength=NBLK).astype(np.int64)
    deg = np.zeros(NPAD, np.float32)
    deg[gmap] = cnt_node
    return {"s": s, "d": dn, "dor": dor, "cnt": cnt, "deg": deg, "gmap": gmap}


def _build_core_arrays(ml, core, TPB, base, NT):
    import ml_dtypes
    x16, W, U, c, b = core["x16"], core["W"], core["U"], core["c"], core["b"]
    s, d, cnt = core["g"]["s"], core["g"]["d"], core["g"]["cnt"]
    dor = core["g"]["dor"]
    E_pad = NT * P
    srcg = np.zeros(E_pad, np.int64)
    dstg = np.zeros(E_pad, np.int64)
    dl = np.full(E_pad, -1.0, np.float32)
    # place each block's edges at its tile base
    cstart = np.concatenate([[0], np.cumsum(cnt)])
    for k in range(NBLK):
        n_k = int(cnt[k])
        if n_k == 0:
            continue
        p0 = base[k] * P
        srcg[p0:p0 + n_k] = s[cstart[k]:cstart[k] + n_k]
        dstg[p0:p0 + n_k] = dor[cstart[k]:cstart[k] + n_k]
        dl[p0:p0 + n_k] = (d[cstart[k]:cstart[k] + n_k] - BLK * k).astype(np.float32)
    xsd = np.empty((P, E_pad), ml_dtypes.bfloat16)
    xsd[:IN_CH, :] = x16[srcg].T
    xsd[IN_CH:, :] = x16[dstg].T
    # one-hot scatter matrix, built on host: oh[p, t*BLK+s] = (dl[t*P+p] == s)
    dlr = dl.reshape(NT, P)
    ohm = (dlr[:, :, None] == np.arange(BLK, dtype=np.float32)[None, None, :])
    ohm = np.ascontiguousarray(
        ohm.transpose(1, 0, 2).reshape(P, NT * BLK)).astype(ml_dtypes.bfloat16)
    # plain layout [h*32+ch]; bottom 64 rows zero (x_dst doesn't enter xjw)
    Wcm = np.zeros((P, P), np.float32)
    Wcm[:IN_CH] = W
    UUc = np.concatenate([U, -U], axis=0)  # [128, 4]
    degp = np.ascontiguousarray(core["g"]["deg"].reshape(NBLK // 2, P).T)  # [128, 98]
    return {
        "xsd": xsd,
        "ohm": ohm,
        "wcm": Wcm.astype(ml_dtypes.bfloat16),
        "uuc": UUc.astype(ml_dtypes.bfloat16),
        "crep": np.tile(c[None, :], (P, 1)).astype(np.float32),
        "brep": np.tile(b[None, :], (P, 1)).astype(np.float32),
        "degp": degp.astype(np.float32),
    }


def _build_program(TPB, NT):
    import concourse.bass as bass
    import concourse.mybir as mybir
    import concourse.bacc as bacc
    from concourse.tile import TileContext

    dt = mybir.dt
    NSC = NT // SCT
    NCH = NT // CH
    # block index / first / last flags per tile
    blk_of = np.repeat(np.arange(NBLK), TPB)
    t0 = np.concatenate([[0], np.cumsum(TPB)])

    nc = bacc.Bacc("TRN2", target_bir_lowering=False, debug=False, num_devices=NCORES)
    xsd_d = nc.dram_tensor("xsd", [P, NT * P], dt.bfloat16, kind="ExternalInput").ap()
    ohm_d = nc.dram_tensor("ohm", [P, NT * BLK], dt.bfloat16, kind="ExternalInput").ap()
    wcm_d = nc.dram_tensor("wcm", [P, P], dt.bfloat16, kind="ExternalInput").ap()
    uuc_d = nc.dram_tensor("uuc", [P, 4], dt.bfloat16, kind="ExternalInput").ap()
    crep_d = nc.dram_tensor("crep", [P, 4], dt.float32, kind="ExternalInput").ap()
    brep_d = nc.dram_tensor("brep", [P, OUT_CH], dt.float32, kind="ExternalInput").ap()
    degp_d = nc.dram_tensor("degp", [P, NBLK // 2], dt.float32, kind="ExternalInput").ap()
    out_d = nc.dram_tensor("out", [NPAD, OUT_CH], dt.float32, kind="ExternalOutput").ap()

    def APn(t, dims, off=0):
        a = t[:]
        return bass.AP(a.tensor, a.offset + off, [a.ap[0]] + dims)

    with TileContext(nc) as tc:
        with tc.tile_pool(name="const", bufs=1) as cp, \
             tc.tile_pool(name="mega", bufs=5) as mp, \
             tc.tile_pool(name="work", bufs=8) as wp, \
             tc.tile_pool(name="qp", bufs=3) as qp, \
             tc.tile_pool(name="fin", bufs=3) as fp, \
             tc.tile_pool(name="finacc", bufs=1) as fap, \
             tc.tile_pool(name="psA", bufs=2, space="PSUM") as psA, \
             tc.tile_pool(name="psU", bufs=1, space="PSUM") as psU, \
             tc.tile_pool(name="psG", bufs=1, space="PSUM") as psG:

            wcm = cp.tile([P, P], dt.bfloat16)
            uuc = cp.tile([P, 4], dt.bfloat16)
            crep = cp.tile([P, 4], dt.float32)
            brep = cp.tile([P, OUT_CH], dt.float32)
            degp = cp.tile([P, NBLK // 2], dt.float32)
            expc = cp.tile([P, 4], dt.float32)
            nc.sync.dma_start(out=wcm[:], in_=wcm_d[:])
            nc.sync.dma_start(out=uuc[:], in_=uuc_d[:])
            nc.sync.dma_start(out=crep[:], in_=crep_d[:])
            nc.sync.dma_start(out=brep[:], in_=brep_d[:])
            nc.sync.dma_start(out=degp[:], in_=degp_d[:])
            nc.scalar.activation(expc[:], crep[:], mybir.ActivationFunctionType.Exp)

            fin = fap.tile([P, (NBLK // 2) * P], dt.float32)

            NH = NBLK // 2
            FIN_BOUNDS = [NH // 4, NH // 2, (3 * NH) // 4, NH]

            def emit_finale(g0, g1):
                ng = g1 - g0
                hs = fp.tile([P, ng * OUT_CH], dt.float32, tag="hs", name="hs")
                nc.vector.tensor_reduce(
                    out=APn(hs, [[32, ng], [1, 32]]),
                    in_=APn(fin, [[P, ng], [1, 32], [32, 4]], off=g0 * P),
                    op=mybir.AluOpType.add, axis=mybir.AxisListType.X)
                dmx = fp.tile([P, ng], dt.float32, tag="dmx", name="dmx")
                nc.vector.tensor_scalar(out=dmx[:], in0=degp[:, g0:g1],
                                        scalar1=1.0, scalar2=None,
                                        op0=mybir.AluOpType.max)
                drc = fp.tile([P, ng], dt.float32, tag="drc", name="drc")
                nc.vector.reciprocal(out=drc[:], in_=dmx[:])
                o1 = fp.tile([P, ng * OUT_CH], dt.float32, tag="o1", name="o1")
                nc.vector.tensor_tensor(
                    out=APn(o1, [[32, ng], [1, 32]]),
                    in0=APn(hs, [[32, ng], [1, 32]]),
                    in1=APn(drc, [[1, ng], [0, 32]]),
                    op=mybir.AluOpType.mult)
                nc.vector.tensor_tensor(
                    out=APn(o1, [[32, ng], [1, 32]]),
                    in0=APn(o1, [[32, ng], [1, 32]]),
                    in1=APn(brep, [[0, ng], [1, 32]]),
                    op=mybir.AluOpType.add)
                o2 = fp.tile([P, ng * OUT_CH], dt.float32, tag="o2", name="o2")
                nc.scalar.mul(o2[:], o1[:], 0.2)
                nc.vector.tensor_tensor(out=o1[:], in0=o1[:], in1=o2[:],
                                        op=mybir.AluOpType.max)
                out_ap = bass.AP(out_d.tensor, out_d.offset + g0 * P * OUT_CH,
                                 [[OUT_CH, P], [P * OUT_CH, ng], [1, OUT_CH]])
                nc.sync.dma_start(out=out_ap, in_=APn(o1, [[32, ng], [1, 32]]))

            acc = None
            xm_t, ohm_t, qe_t = {}, {}, {}

            def emit_dma(sc):
                xm = mp.tile([P, SCT * P], dt.bfloat16, tag="xm", name="xm")
                nc.sync.dma_start(out=xm[:], in_=xsd_d[:, sc * SCT * P:(sc + 1) * SCT * P])
                ohm = mp.tile([P, SCT * BLK], dt.bfloat16, tag="ohm", name="ohm")
                nc.sync.dma_start(out=ohm[:], in_=ohm_d[:, sc * SCT * BLK:(sc + 1) * SCT * BLK])
                xm_t[sc] = xm
                ohm_t[sc] = ohm

            def emit_pu_softmax(sc):
                # pU matmuls + softmax for superchunk sc, emitted one SC ahead
                # of its pA/scatter work so the exp->softmax->stg chain is off
                # the tensor engine's critical path
                xm = xm_t[sc]
                pU = psU.tile([P, SCT * 4], dt.float32, tag="pU", name="pU")
                qe = qp.tile([P, SCT * 4], dt.float32, tag="qe", name="qe")
                qb = qp.tile([P, SCT * 4], dt.float32, tag="qb", name="qb")
                den = qp.tile([P, SCT], dt.float32, tag="den", name="den")
                rec = qp.tile([P, SCT], dt.float32, tag="rec", name="rec")
                for j in range(SCT):
                    nc.tensor.matmul(out=pU[:, j * 4:(j + 1) * 4],
                                     lhsT=xm[:, j * P:(j + 1) * P], rhs=uuc[:],
                                     start=True, stop=True)
                nc.scalar.activation(qe[:], pU[:], mybir.ActivationFunctionType.Exp)
                # softmax bookkeeping on gpsimd (idle engine); reciprocal has
                # no gpsimd equivalent so it stays on vector
                nc.gpsimd.tensor_tensor(
                    out=APn(qb, [[4, SCT], [1, 4]]),
                    in0=APn(qe, [[4, SCT], [1, 4]]),
                    in1=APn(expc, [[0, SCT], [1, 4]]),
                    op=mybir.AluOpType.mult)
                nc.vector.tensor_reduce(
                    out=den[:], in_=APn(qb, [[4, SCT], [1, 4]]),
                    op=mybir.AluOpType.add, axis=mybir.AxisListType.X)
                nc.vector.reciprocal(out=rec[:], in_=den[:])
                nc.gpsimd.tensor_tensor(
                    out=APn(qe, [[4, SCT], [1, 4]]),
                    in0=APn(qb, [[4, SCT], [1, 4]]),
                    in1=APn(rec, [[1, SCT], [0, 4]]),
                    op=mybir.AluOpType.mult)
                qe_t[sc] = qe

            def emit_chunk(sc, ci):
                nonlocal acc
                xm, ohm, qe = xm_t[sc], ohm_t[sc], qe_t[sc]
                pA = psA.tile([P, CH * P], dt.float32, tag="pA", name="pA")
                for i in range(CH):
                    lhs = xm[:, (ci * CH + i) * P:(ci * CH + i + 1) * P]
                    nc.tensor.matmul(out=pA[:, i * P:(i + 1) * P], lhsT=lhs,
                                     rhs=wcm[:], start=True, stop=True)
                stg = wp.tile([P, CH * P], dt.bfloat16, tag="stg", name="stg")
                CH2 = CH // 2
                for hh in range(2):
                    # z~ = q * xjw, in half-chunks so the first half's scatter
                    # matmuls overlap the second half's multiply
                    nc.vector.tensor_tensor(
                        out=APn(stg, [[P, CH2], [32, 4], [1, 32]], off=hh * CH2 * P),
                        in0=APn(pA, [[P, CH2], [32, 4], [1, 32]], off=hh * CH2 * P),
                        in1=APn(qe, [[4, CH2], [1, 4], [0, 32]],
                                off=(ci * CH + hh * CH2) * 4),
                        op=mybir.AluOpType.mult)
                    for i in range(hh * CH2, (hh + 1) * CH2):
                        t = sc * SCT + ci * CH + i
                        k = int(blk_of[t])
                        if k % 2 == 0 and t == t0[k]:
                            acc = psG.tile([P, P], dt.float32, tag="acc", name="acc")
                        half = (k % 2) * BLK
                        nc.tensor.matmul(
                            out=acc[half:half + BLK, :],
                            lhsT=ohm[:, (ci * CH + i) * BLK:(ci * CH + i + 1) * BLK],
                            rhs=stg[:, i * P:(i + 1) * P],
                            start=(t == t0[k]), stop=(t == t0[k + 1] - 1))
                        if k % 2 == 1 and t == t0[k + 1] - 1:
                            m = k // 2
                            nc.scalar.copy(out=fin[:, m * P:(m + 1) * P], in_=acc[:])
                            if (m + 1) in FIN_BOUNDS:
                                emit_finale(FIN_BOUNDS[FIN_BOUNDS.index(m + 1) - 1]
                                            if FIN_BOUNDS.index(m + 1) > 0 else 0,
                                            m + 1)

            emit_dma(0)
            if NSC > 1:
                emit_dma(1)
            emit_pu_softmax(0)
            for sc in range(NSC):
                if sc + 2 < NSC:
                    emit_dma(sc + 2)
                emit_chunk(sc, 0)
                # pU matmuls for the next SC fill the tensor queue while this
                # SC's first stg multiply runs on the vector engine
                if sc + 1 < NSC:
                    emit_pu_softmax(sc + 1)
                for ci in range(1, SCT // CH):
                    emit_chunk(sc, ci)
                xm_t.pop(sc), ohm_t.pop(sc), qe_t.pop(sc)
    nc.compile()
    return nc


def kernel(x_v, edge_index_v, x_f, edge_index_f, Wv, Uv, cv, bv, Wf, Uf, cf, bf):
    _register_ntff_hook()
    import ml_dtypes
    from concourse import bass_utils

    x_v = np.asarray(x_v, np.float32)
    x_f = np.asarray(x_f, np.float32)
    cores = []
    for bi, (x, ei, W, U, c, b) in enumerate([
            (x_v, edge_index_v, Wv, Uv, cv, bv),
            (x_f, edge_index_f, Wf, Uf, cf, bf)]):
        ei = np.asarray(ei)
        s0, d0 = ei[0].astype(np.int64), ei[1].astype(np.int64)
        m = s0 != d0
        loops = np.arange(N, dtype=np.int64)
        src = np.concatenate([s0[m], loops])
        dst = np.concatenate([d0[m], loops])
        x16 = x.astype(ml_dtypes.bfloat16)
        for j in range(4):
            lo = j * NPC
            cores.append({
                "x16": x16, "W": np.asarray(W, np.float32),
                "U": np.asarray(U, np.float32), "c": np.asarray(c, np.float32),
                "b": np.asarray(b, np.float32), "lo": lo,
                "g": _prep_core(x16, src, dst, lo),
            })

    tn = np.stack([np.ceil(c["g"]["cnt"] / P).astype(np.int64) for c in cores])
    TPB = tn.max(axis=0)
    TPB = np.maximum(TPB, 1)
    NT = int(TPB.sum())
    pad = (-NT) % SCT
    TPB[NBLK - 1] += pad
    NT += pad
    base = np.concatenate([[0], np.cumsum(TPB)])[:-1]

    in_maps = []
    for c in cores:
        arrs = _build_core_arrays(None, c, TPB, base, NT)
        in_maps.append(arrs)

    nc = _build_program(TPB, NT)
    res = bass_utils.run_bass_kernel_spmd(
        nc, in_maps, core_ids=list(range(NCORES)),
        trace=bool(int(__import__("os").environ.get("KERNEL_TRACE", "0"))))
    kernel.last_result = res
    out_v = np.concatenate(
        [res.results[j]["out"][cores[j]["g"]["gmap"]] for j in range(4)])
    out_f = np.concatenate(
        [res.results[4 + j]["out"][cores[4 + j]["g"]["gmap"]] for j in range(4)])
    return out_v, out_f



# revision 31
# speedup vs baseline: 1.0811x; 1.0811x over previous
"""FeaStConv dual-branch GNN message passing on 8 Trainium2 NeuronCores.

Sharding: branch v on cores 0-3, branch f on cores 4-7; each core owns a
12500-node destination range. Host reorders edges by destination block
(64 nodes), pre-gathers transposed source/dest features (bf16), device does
all float math: per-tile matmuls for x@W / (x_s-x_d)@U, softmax on-device,
one-hot scatter matmuls accumulating per-block in PSUM.
"""
import sys, types
import numpy as np

sys.path.insert(0, '/opt/trn_rl_repo')

N = 50000
IN_CH = 64
HEADS = 4
OUT_CH = 32
P = 128
NPC = 12500           # nodes per core
BLK = 64              # dst nodes per block
NBLK = 210            # blocks per core (210*64 = 13440)
NPAD = NBLK * BLK
BLK_EDGE_CAP = 1024   # pack blocks to <= 8 tiles of edges where possible
CH = 12              # tiles per chunk
SCT = 36              # tiles per superchunk
NCORES = 8


def _register_ntff_hook():
    import antenv
    if "antenv.axon_hooks" in sys.modules:
        return
    mod = types.ModuleType("antenv.axon_hooks")
    _h = [None]
    mod.set_axon_ntff_profile_hook = lambda h: _h.__setitem__(0, h)
    mod.get_axon_ntff_profile_hook = lambda: _h[0]
    sys.modules["antenv.axon_hooks"] = mod
    antenv.axon_hooks = mod
    if "/root/.axon_site" not in sys.path:
        sys.path.insert(0, "/root/.axon_site")
    try:
        from trn_agent_boot.trn_boot import _ntff_profile_via_ctypes
        mod.set_axon_ntff_profile_hook(_ntff_profile_via_ctypes('/opt/axon/libaxon_pjrt.so'))
    except Exception:
        pass


def _pack_blocks(cnt_node):
    """Assign each of the NPC destination nodes to one of NBLK 64-node blocks,
    packing so block edge totals stay <= BLK_EDGE_CAP (8 tiles) where
    possible; overflow concentrates in the highest-index blocks so the
    max-over-cores tile profile stays tight. Returns gmap[node] -> padded
    node id (block*BLK + slot)."""
    order = np.argsort(-cnt_node, kind='stable')
    bsum = np.zeros(NBLK, np.int64)
    bcnt = np.zeros(NBLK, np.int64)
    gmap = np.empty(NPC, np.int64)
    idx = np.arange(NBLK)
    for n in order:
        c = int(cnt_node[n])
        fit = (bcnt < BLK) & (bsum + c <= BLK_EDGE_CAP)
        if fit.any():
            # best-fit: fullest block that still fits
            b = int(np.argmax(np.where(fit, bsum, -1)))
        else:
            room = bcnt < BLK
            b = int(idx[room][-1])  # spill into highest-index open block
        gmap[n] = b * BLK + bcnt[b]
        bcnt[b] += 1
        bsum[b] += c
    return gmap


def _prep_core(x16, src, dst, lo):
    """Per-core edge layout. Returns dict with per-block counts and sorted
    (global-src, packed-local-dst, original-global-dst) arrays."""
    sel = (dst >= lo) & (dst < lo + NPC)
    s = src[sel]
    d0 = (dst[sel] - lo).astype(np.int64)
    cnt_node = np.bincount(d0, minlength=NPC).astype(np.int64)
    gmap = _pack_blocks(cnt_node)
    dn = gmap[d0]
    order = np.argsort(dn, kind='stable')
    s = s[order]
    dn = dn[order]
    dor = d0[order] + lo
    cnt = np.bincount(dn >> 6, minlength=NBLK).astype(np.int64)
    deg = np.zeros(NPAD, np.float32)
    deg[gmap] = cnt_node
    return {"s": s, "d": dn, "dor": dor, "cnt": cnt, "deg": deg, "gmap": gmap}


def _build_core_arrays(ml, core, TPB, base, NT):
    import ml_dtypes
    x16, W, U, c, b = core["x16"], core["W"], core["U"], core["c"], core["b"]
    s, d, cnt = core["g"]["s"], core["g"]["d"], core["g"]["cnt"]
    dor = core["g"]["dor"]
    E_pad = NT * P
    srcg = np.zeros(E_pad, np.int64)
    dstg = np.zeros(E_pad, np.int64)
    dl = np.full(E_pad, -1.0, np.float32)
    # place each block's edges at its tile base
    cstart = np.concatenate([[0], np.cumsum(cnt)])
    for k in range(NBLK):
        n_k = int(cnt[k])
        if n_k == 0:
            continue
        p0 = base[k] * P
        srcg[p0:p0 + n_k] = s[cstart[k]:cstart[k] + n_k]
        dstg[p0:p0 + n_k] = dor[cstart[k]:cstart[k] + n_k]
        dl[p0:p0 + n_k] = (d[cstart[k]:cstart[k] + n_k] - BLK * k).astype(np.float32)
    xsd = np.empty((P, E_pad), ml_dtypes.bfloat16)
    xsd[:IN_CH, :] = x16[srcg].T
    xsd[IN_CH:, :] = x16[dstg].T
    # one-hot scatter matrix, built on host: oh[p, t*BLK+s] = (dl[t*P+p] == s)
    dlr = dl.reshape(NT, P)
    ohm = (dlr[:, :, None] == np.arange(BLK, dtype=np.float32)[None, None, :])
    ohm = np.ascontiguousarray(
        ohm.transpose(1, 0, 2).reshape(P, NT * BLK)).astype(ml_dtypes.bfloat16)
    # plain layout [h*32+ch]; bottom 64 rows zero (x_dst doesn't enter xjw)
    Wcm = np.zeros((P, P), np.float32)
    Wcm[:IN_CH] = W
    UUc = np.concatenate([U, -U], axis=0)  # [128, 4]
    degp = np.ascontiguousarray(core["g"]["deg"].reshape(NBLK // 2, P).T)  # [128, 98]
    return {
        "xsd": xsd,
        "ohm": ohm,
        "wcm": Wcm.astype(ml_dtypes.bfloat16),
        "uuc": UUc.astype(ml_dtypes.bfloat16),
        "crep": np.tile(c[None, :], (P, 1)).astype(np.float32),
        "brep": np.tile(b[None, :], (P, 1)).astype(np.float32),
        "degp": degp.astype(np.float32),
    }


def _build_program(TPB, NT):
    import concourse.bass as bass
    import concourse.mybir as mybir
    import concourse.bacc as bacc
    from concourse.tile import TileContext

    dt = mybir.dt
    NSC = NT // SCT
    NCH = NT // CH
    # block index / first / last flags per tile
    blk_of = np.repeat(np.arange(NBLK), TPB)
    t0 = np.concatenate([[0], np.cumsum(TPB)])

    nc = bacc.Bacc("TRN2", target_bir_lowering=False, debug=False, num_devices=NCORES)
    xsd_d = nc.dram_tensor("xsd", [P, NT * P], dt.bfloat16, kind="ExternalInput").ap()
    ohm_d = nc.dram_tensor("ohm", [P, NT * BLK], dt.bfloat16, kind="ExternalInput").ap()
    wcm_d = nc.dram_tensor("wcm", [P, P], dt.bfloat16, kind="ExternalInput").ap()
    uuc_d = nc.dram_tensor("uuc", [P, 4], dt.bfloat16, kind="ExternalInput").ap()
    crep_d = nc.dram_tensor("crep", [P, 4], dt.float32, kind="ExternalInput").ap()
    brep_d = nc.dram_tensor("brep", [P, OUT_CH], dt.float32, kind="ExternalInput").ap()
    degp_d = nc.dram_tensor("degp", [P, NBLK // 2], dt.float32, kind="ExternalInput").ap()
    out_d = nc.dram_tensor("out", [NPAD, OUT_CH], dt.float32, kind="ExternalOutput").ap()

    def APn(t, dims, off=0):
        a = t[:]
        return bass.AP(a.tensor, a.offset + off, [a.ap[0]] + dims)

    with TileContext(nc) as tc:
        with tc.tile_pool(name="const", bufs=1) as cp, \
             tc.tile_pool(name="mega", bufs=4) as mp, \
             tc.tile_pool(name="work", bufs=8) as wp, \
             tc.tile_pool(name="qp", bufs=3) as qp, \
             tc.tile_pool(name="fin", bufs=3) as fp, \
             tc.tile_pool(name="finacc", bufs=1) as fap, \
             tc.tile_pool(name="psA", bufs=2, space="PSUM") as psA, \
             tc.tile_pool(name="psU", bufs=1, space="PSUM") as psU, \
             tc.tile_pool(name="psG", bufs=1, space="PSUM") as psG:

            wcm = cp.tile([P, P], dt.bfloat16)
            uuc = cp.tile([P, 4], dt.bfloat16)
            crep = cp.tile([P, 4], dt.float32)
            brep = cp.tile([P, OUT_CH], dt.float32)
            degp = cp.tile([P, NBLK // 2], dt.float32)
            expc = cp.tile([P, 4], dt.float32)
            nc.sync.dma_start(out=wcm[:], in_=wcm_d[:])
            nc.sync.dma_start(out=uuc[:], in_=uuc_d[:])
            nc.sync.dma_start(out=crep[:], in_=crep_d[:])
            nc.sync.dma_start(out=brep[:], in_=brep_d[:])
            nc.sync.dma_start(out=degp[:], in_=degp_d[:])
            nc.scalar.activation(expc[:], crep[:], mybir.ActivationFunctionType.Exp)

            fin = fap.tile([P, (NBLK // 2) * P], dt.float32)

            NH = NBLK // 2
            FIN_BOUNDS = [NH // 4, NH // 2, (3 * NH) // 4, NH]

            def emit_finale(g0, g1):
                ng = g1 - g0
                hs = fp.tile([P, ng * OUT_CH], dt.float32, tag="hs", name="hs")
                nc.vector.tensor_reduce(
                    out=APn(hs, [[32, ng], [1, 32]]),
                    in_=APn(fin, [[P, ng], [1, 32], [32, 4]], off=g0 * P),
                    op=mybir.AluOpType.add, axis=mybir.AxisListType.X)
                dmx = fp.tile([P, ng], dt.float32, tag="dmx", name="dmx")
                nc.vector.tensor_scalar(out=dmx[:], in0=degp[:, g0:g1],
                                        scalar1=1.0, scalar2=None,
                                        op0=mybir.AluOpType.max)
                drc = fp.tile([P, ng], dt.float32, tag="drc", name="drc")
                nc.vector.reciprocal(out=drc[:], in_=dmx[:])
                o1 = fp.tile([P, ng * OUT_CH], dt.float32, tag="o1", name="o1")
                nc.vector.tensor_tensor(
                    out=APn(o1, [[32, ng], [1, 32]]),
                    in0=APn(hs, [[32, ng], [1, 32]]),
                    in1=APn(drc, [[1, ng], [0, 32]]),
                    op=mybir.AluOpType.mult)
                nc.vector.tensor_tensor(
                    out=APn(o1, [[32, ng], [1, 32]]),
                    in0=APn(o1, [[32, ng], [1, 32]]),
                    in1=APn(brep, [[0, ng], [1, 32]]),
                    op=mybir.AluOpType.add)
                o2 = fp.tile([P, ng * OUT_CH], dt.float32, tag="o2", name="o2")
                nc.scalar.mul(o2[:], o1[:], 0.2)
                nc.vector.tensor_tensor(out=o1[:], in0=o1[:], in1=o2[:],
                                        op=mybir.AluOpType.max)
                out_ap = bass.AP(out_d.tensor, out_d.offset + g0 * P * OUT_CH,
                                 [[OUT_CH, P], [P * OUT_CH, ng], [1, OUT_CH]])
                nc.sync.dma_start(out=out_ap, in_=APn(o1, [[32, ng], [1, 32]]))

            acc = None
            xm_t, ohm_t, qe_t = {}, {}, {}

            def emit_dma(sc):
                xm = mp.tile([P, SCT * P], dt.bfloat16, tag="xm", name="xm")
                nc.sync.dma_start(out=xm[:], in_=xsd_d[:, sc * SCT * P:(sc + 1) * SCT * P])
                ohm = mp.tile([P, SCT * BLK], dt.bfloat16, tag="ohm", name="ohm")
                nc.sync.dma_start(out=ohm[:], in_=ohm_d[:, sc * SCT * BLK:(sc + 1) * SCT * BLK])
                xm_t[sc] = xm
                ohm_t[sc] = ohm

            def emit_pu_softmax(sc):
                # pU matmuls + softmax for superchunk sc, emitted one SC ahead
                # of its pA/scatter work so the exp->softmax->stg chain is off
                # the tensor engine's critical path
                xm = xm_t[sc]
                pU = psU.tile([P, SCT * 4], dt.float32, tag="pU", name="pU")
                qe = qp.tile([P, SCT * 4], dt.float32, tag="qe", name="qe")
                qb = qp.tile([P, SCT * 4], dt.float32, tag="qb", name="qb")
                den = qp.tile([P, SCT], dt.float32, tag="den", name="den")
                rec = qp.tile([P, SCT], dt.float32, tag="rec", name="rec")
                for j in range(SCT):
                    nc.tensor.matmul(out=pU[:, j * 4:(j + 1) * 4],
                                     lhsT=xm[:, j * P:(j + 1) * P], rhs=uuc[:],
                                     start=True, stop=True)
                nc.scalar.activation(qe[:], pU[:], mybir.ActivationFunctionType.Exp)
                # softmax bookkeeping on gpsimd (idle engine); reciprocal has
                # no gpsimd equivalent so it stays on vector
                nc.gpsimd.tensor_tensor(
                    out=APn(qb, [[4, SCT], [1, 4]]),
                    in0=APn(qe, [[4, SCT], [1, 4]]),
                    in1=APn(expc, [[0, SCT], [1, 4]]),
                    op=mybir.AluOpType.mult)
                nc.vector.tensor_reduce(
                    out=den[:], in_=APn(qb, [[4, SCT], [1, 4]]),
                    op=mybir.AluOpType.add, axis=mybir.AxisListType.X)
                nc.vector.reciprocal(out=rec[:], in_=den[:])
                nc.gpsimd.tensor_tensor(
                    out=APn(qe, [[4, SCT], [1, 4]]),
                    in0=APn(qb, [[4, SCT], [1, 4]]),
                    in1=APn(rec, [[1, SCT], [0, 4]]),
                    op=mybir.AluOpType.mult)
                qe_t[sc] = qe

            def emit_chunk(sc, ci):
                nonlocal acc
                xm, ohm, qe = xm_t[sc], ohm_t[sc], qe_t[sc]
                pA = psA.tile([P, CH * P], dt.float32, tag="pA", name="pA")
                for i in range(CH):
                    lhs = xm[:, (ci * CH + i) * P:(ci * CH + i + 1) * P]
                    nc.tensor.matmul(out=pA[:, i * P:(i + 1) * P], lhsT=lhs,
                                     rhs=wcm[:], start=True, stop=True)
                stg = wp.tile([P, CH * P], dt.bfloat16, tag="stg", name="stg")
                # z~ = q * xjw  (plain layout: col i*128 + h*32 + ch)
                nc.vector.tensor_tensor(
                    out=APn(stg, [[P, CH], [32, 4], [1, 32]]),
                    in0=APn(pA, [[P, CH], [32, 4], [1, 32]]),
                    in1=APn(qe, [[4, CH], [1, 4], [0, 32]], off=ci * CH * 4),
                    op=mybir.AluOpType.mult)
                for i in range(CH):
                    t = sc * SCT + ci * CH + i
                    k = int(blk_of[t])
                    if k % 2 == 0 and t == t0[k]:
                        acc = psG.tile([P, P], dt.float32, tag="acc", name="acc")
                    half = (k % 2) * BLK
                    nc.tensor.matmul(
                        out=acc[half:half + BLK, :],
                        lhsT=ohm[:, (ci * CH + i) * BLK:(ci * CH + i + 1) * BLK],
                        rhs=stg[:, i * P:(i + 1) * P],
                        start=(t == t0[k]), stop=(t == t0[k + 1] - 1))
                    if k % 2 == 1 and t == t0[k + 1] - 1:
                        m = k // 2
                        nc.scalar.copy(out=fin[:, m * P:(m + 1) * P], in_=acc[:])
                        if (m + 1) in FIN_BOUNDS:
                            emit_finale(FIN_BOUNDS[FIN_BOUNDS.index(m + 1) - 1]
                                        if FIN_BOUNDS.index(m + 1) > 0 else 0,
                                        m + 1)

            emit_dma(0)
            if NSC > 1:
                emit_dma(1)
            emit_pu_softmax(0)
            for sc in range(NSC):
                if sc + 2 < NSC:
                    emit_dma(sc + 2)
                emit_chunk(sc, 0)
                # pU matmuls for the next SC fill the tensor queue while this
                # SC's first stg multiply runs on the vector engine
                if sc + 1 < NSC:
                    emit_pu_softmax(sc + 1)
                for ci in range(1, SCT // CH):
                    emit_chunk(sc, ci)
                xm_t.pop(sc), ohm_t.pop(sc), qe_t.pop(sc)
    nc.compile()
    return nc


def kernel(x_v, edge_index_v, x_f, edge_index_f, Wv, Uv, cv, bv, Wf, Uf, cf, bf):
    _register_ntff_hook()
    import ml_dtypes
    from concourse import bass_utils

    x_v = np.asarray(x_v, np.float32)
    x_f = np.asarray(x_f, np.float32)
    cores = []
    for bi, (x, ei, W, U, c, b) in enumerate([
            (x_v, edge_index_v, Wv, Uv, cv, bv),
            (x_f, edge_index_f, Wf, Uf, cf, bf)]):
        ei = np.asarray(ei)
        s0, d0 = ei[0].astype(np.int64), ei[1].astype(np.int64)
        m = s0 != d0
        loops = np.arange(N, dtype=np.int64)
        src = np.concatenate([s0[m], loops])
        dst = np.concatenate([d0[m], loops])
        x16 = x.astype(ml_dtypes.bfloat16)
        for j in range(4):
            lo = j * NPC
            cores.append({
                "x16": x16, "W": np.asarray(W, np.float32),
                "U": np.asarray(U, np.float32), "c": np.asarray(c, np.float32),
                "b": np.asarray(b, np.float32), "lo": lo,
                "g": _prep_core(x16, src, dst, lo),
            })

    tn = np.stack([np.ceil(c["g"]["cnt"] / P).astype(np.int64) for c in cores])
    TPB = tn.max(axis=0)
    TPB = np.maximum(TPB, 1)
    NT = int(TPB.sum())
    pad = (-NT) % SCT
    TPB[NBLK - 1] += pad
    NT += pad
    base = np.concatenate([[0], np.cumsum(TPB)])[:-1]

    in_maps = []
    for c in cores:
        arrs = _build_core_arrays(None, c, TPB, base, NT)
        in_maps.append(arrs)

    nc = _build_program(TPB, NT)
    res = bass_utils.run_bass_kernel_spmd(
        nc, in_maps, core_ids=list(range(NCORES)),
        trace=bool(int(__import__("os").environ.get("KERNEL_TRACE", "0"))))
    kernel.last_result = res
    out_v = np.concatenate(
        [res.results[j]["out"][cores[j]["g"]["gmap"]] for j in range(4)])
    out_f = np.concatenate(
        [res.results[4 + j]["out"][cores[4 + j]["g"]["gmap"]] for j in range(4)])
    return out_v, out_f

